# revision 1
# baseline (speedup 1.0000x reference)
"""Chamfer loss (+ jet 4-momentum term) on 8 Trainium2 NeuronCores.

Problem: p, q: (64, 2048, 4) fp32.
  loss = sum_b [ sum_i min_j d(i,j) + sum_j min_i d(i,j) ] + |sum_n p - sum_n q|^2
  with d(i,j) = |p_i - q_j|^2.

Strategy (data-parallel over batch, 8 batches/core):
  - Host: augment features so ONE K=6 bf16 matmul yields the negated distance
    matrix directly:
      pt = -[p0..p3, |p|^2, 1]^T  (6, N),  qt = [-2q0..-2q3, 1, |q|^2]^T (6, N)
      -dist = pt^T @ qt  (fp32 PSUM; bf16 inputs — |total err| ~3e-3 rel,
      tolerance is 2e-2)
  - Per batch, 16 row-blocks of 128: 4 matmuls/block into a [128,2048] PSUM
    tile; ACT drains each block to a bf16 SBUF grid (plain copy, the negation
    is baked into pt).  Mins become maxes on the negated values:
      row path: balanced in-place max-fold tree over j + one X-reduce
      col path: DVE fold 8->4, Pool folds 4->2->1, merge, Pool
                partition_all_reduce(max), ACT accum-sum over j
    Engine balance per batch: ACT ~32us (drains), DVE ~29us, Pool ~21us.
  - Jet term: one ones-matmul over a host-pretiled (p-q) layout.
  - reps (for wall-time slope benching) is a HARDWARE For_i loop around the
    per-rep body, so the NEFF size is constant in reps and the measured slope
    is pure device time per rep.
  - Final: per-core scalar via matmul-with-ones; host adds the 8 partials
    (the "all-reduce").
"""

import numpy as np

B, N, D = 64, 2048, 4
N_CORES = 8
BPC = B // N_CORES  # batches per core
NB = N // 128       # 128-row blocks per batch
HB = NB // 2        # blocks per half-batch

_cache: dict = {}


def _build_bass(reps: int = 1):
    import concourse.bacc as bacc
    import concourse.tile as tile
    from concourse import mybir
    from concourse import bass_isa

    f32 = mybir.dt.float32
    bf16 = mybir.dt.bfloat16
    MAX = mybir.AluOpType.max
    ADD = mybir.AluOpType.add
    X = mybir.AxisListType.X
    COPY = mybir.ActivationFunctionType.Copy

    nc = bacc.Bacc(None, target_bir_lowering=False)
    pt_d = nc.declare_dram_parameter("pt", [BPC, 6, N], bf16, isOutput=False)
    qt_d = nc.declare_dram_parameter("qt", [BPC, 6, N], bf16, isOutput=False)
    jq_d = nc.declare_dram_parameter("jq", [128, BPC * 64], f32, isOutput=False)
    out_d = nc.declare_dram_parameter("out", [1, 1], f32, isOutput=True)

    with tile.TileContext(nc) as tc:
        with (
            tc.tile_pool(name="consts", bufs=1) as consts,
            tc.tile_pool(name="io", bufs=2) as io,
            tc.tile_pool(name="gridp", bufs=2) as gridp,
            tc.tile_pool(name="rowp", bufs=1) as rowp,
            tc.tile_pool(name="s4p", bufs=2) as s4p,
            tc.tile_pool(name="cmp", bufs=2) as cmp,
            tc.tile_pool(name="minp", bufs=1) as minp,
            tc.tile_pool(name="scr", bufs=2) as scr,
            tc.tile_pool(name="psum", bufs=2, space="PSUM") as psum,
        ):
            ones = consts.tile([128, 1], f32)
            nc.vector.memset(ones, 1.0)
            jqt = consts.tile([128, BPC * 64], f32)
            nc.sync.dma_start(out=jqt, in_=jq_d[:, :])

            rowmax = minp.tile([128, BPC * NB], bf16)  # per-block row maxes of -dist
            # per-batch partition-collapsed col maxes (broadcast rows); summed
            # once in the epilogue so no per-batch op ever waits on the Pool chain
            colredall = minp.tile([128, BPC, N], bf16)

            with tc.For_i(0, reps, 1, hint_engines=(mybir.EngineType.PE,)):
                for b in range(BPC):
                    pt = io.tile([6, N], bf16, tag="pt")
                    qt = io.tile([6, N], bf16, tag="qt")
                    nc.sync.dma_start(out=pt, in_=pt_d[b])
                    nc.sync.dma_start(out=qt, in_=qt_d[b])

                    colmin2 = cmp.tile([128, 2, N], bf16, tag="cm2")
                    for h in range(2):
                        grid = gridp.tile([128, HB, N], bf16, tag="grid")
                        for t8 in range(HB):
                            t = h * HB + t8
                            d_ps = psum.tile([128, N], f32, tag="d")
                            lhsT = pt[:, t * 128 : (t + 1) * 128]
                            for c in range(4):
                                nc.tensor.matmul(
                                    d_ps[:, c * 512 : (c + 1) * 512],
                                    lhsT,
                                    qt[:, c * 512 : (c + 1) * 512],
                                    start=True,
                                    stop=True,
                                )
                            # drain -dist (negation baked into pt) -> bf16 grid
                            nc.scalar.activation(out=grid[:, t8, :], in_=d_ps, func=COPY)

                        # col path: fold blocks 8->4->2->1 (DVE; Pool's ISA has
                        # no 2-input tensor op, backend NCC_IXCG966)
                        s4 = s4p.tile([128, 4, N], bf16, tag="s4")
                        nc.vector.tensor_tensor(s4, grid[:, 0:4, :], grid[:, 4:8, :], MAX)
                        nc.vector.tensor_tensor(s4[:, 0:2, :], s4[:, 0:2, :], s4[:, 2:4, :], MAX)
                        nc.vector.tensor_tensor(colmin2[:, h, :], s4[:, 0, :], s4[:, 1, :], MAX)

                        # row path: j-fold tree into rowscr (grid stays
                        # read-only -> its slot frees early; bufs=1 is fine, the
                        # WAR with the previous half is DVE-in-order anyway),
                        # then X-reduce to [128, HB]
                        rowscr = rowp.tile([128, HB, 1024], bf16, tag="rowscr")
                        nc.vector.tensor_tensor(
                            rowscr, grid[:, :, 0:1024], grid[:, :, 1024:2048], MAX
                        )
                        nc.vector.tensor_tensor(
                            rowscr[:, :, 0:512], rowscr[:, :, 0:512], rowscr[:, :, 512:1024], MAX
                        )
                        nc.vector.tensor_tensor(
                            rowscr[:, :, 0:256], rowscr[:, :, 0:256], rowscr[:, :, 256:512], MAX
                        )
                        nc.vector.tensor_tensor(
                            rowscr[:, :, 0:128], rowscr[:, :, 0:128], rowscr[:, :, 128:256], MAX
                        )
                        nc.vector.tensor_reduce(
                            out=rowmax[:, b * NB + h * HB : b * NB + (h + 1) * HB],
                            in_=rowscr[:, :, 0:128],
                            axis=X,
                            op=MAX,
                        )

                    # merge halves, collapse partitions into the slab (Pool)
                    colmin = cmp.tile([128, N], bf16, tag="cm")
                    nc.vector.tensor_tensor(colmin, colmin2[:, 0, :], colmin2[:, 1, :], MAX)
                    nc.gpsimd.partition_all_reduce(
                        colredall[:, b, :], colmin, 128, bass_isa.ReduceOp.max
                    )
                    if b >= BPC // 2:
                        # fold the colred slab pairwise in-loop so the epilogue
                        # tail (after the last drain) stays short
                        bl = b - BPC // 2
                        nc.vector.tensor_tensor(
                            colredall[:, bl, :], colredall[:, bl, :],
                            colredall[:, b, :], ADD,
                        )

            # epilogue: total = -(sum(rowmax) + sum_b sum_j colredall[b])
            #                   + sum(jd^2)
            # finish the batch-dim sum tree (rows are broadcast-equal)
            nc.vector.tensor_tensor(
                colredall[:, 0:2, :], colredall[:, 0:2, :], colredall[:, 2:4, :], ADD
            )
            nc.vector.tensor_tensor(
                colredall[:, 0:1, :], colredall[:, 0:1, :], colredall[:, 1:2, :], ADD
            )
            r1 = scr.tile([128, 1], f32, tag="r1")
            ctot = scr.tile([128, 1], f32, tag="ctot")
            nc.vector.tensor_reduce(out=r1, in_=rowmax, axis=X, op=ADD)
            nc.vector.tensor_reduce(out=ctot, in_=colredall[:, 0, :], axis=X, op=ADD)
            nc.vector.tensor_add(r1[0:1, :], r1[0:1, :], ctot[0:1, :])
            nc.vector.tensor_scalar_mul(r1, r1, -1.0)

            # jet: jd[b,d] = sum_n (p - q) via ones-matmul over the partition dim,
            # then square+sum; jq columns are (b, d, chunk) with n = chunk*128 + r
            jps = psum.tile([1, BPC * 64], f32, tag="d")
            nc.tensor.matmul(jps, ones, jqt, start=True, stop=True)
            jdr = scr.tile([1, BPC * 4], f32, tag="jdr")
            nc.vector.tensor_reduce(
                out=jdr,
                in_=jps.rearrange("p (b d c) -> p (b d) c", b=BPC, c=16, d=4),
                axis=X,
                op=ADD,
            )
            jd2 = scr.tile([1, BPC * 4], f32, tag="jd2")
            jtot = scr.tile([1, 1], f32, tag="jtot")
            nc.vector.tensor_mul(jd2, jdr, jdr)
            nc.vector.tensor_reduce(out=jtot, in_=jd2, axis=X, op=ADD)
            nc.vector.tensor_add(r1[0:1, :], r1[0:1, :], jtot)

            fin_ps = psum.tile([128, N], f32, tag="d")
            fin = fin_ps[0:1, 0:1]
            nc.tensor.matmul(fin, r1, ones, start=True, stop=True)
            out_sb = scr.tile([1, 1], f32, tag="out")
            nc.vector.tensor_copy(out=out_sb, in_=fin)
            nc.sync.dma_start(out=out_d[:, :], in_=out_sb)

    nc.compile()
    return nc


def _augment(p: np.ndarray, q: np.ndarray):
    """K=6 augmented features, negation baked into pt:

    pt = -[p0..p3, |p|^2, 1]^T, qt = [-2q0..-2q3, 1, |q|^2]^T (both bf16), so
    pt^T @ qt = -dist and every min becomes a max on the device.
    """
    import ml_dtypes

    bf = ml_dtypes.bfloat16
    Bn = p.shape[0]
    pt = np.empty((Bn, 6, N), np.float32)
    pt[:, 0:4] = -p.transpose(0, 2, 1)
    pt[:, 4] = -np.square(p).sum(-1)
    pt[:, 5] = -1.0
    qt = np.empty((Bn, 6, N), np.float32)
    qt[:, 0:4] = (-2.0 * q).transpose(0, 2, 1)
    qt[:, 4] = 1.0
    qt[:, 5] = np.square(q).sum(-1)
    pt_s = pt.astype(bf)
    qt_s = qt.astype(bf)
    # jet input: (128, B*64) with col = b*64 + d*16 + chunk, n = chunk*128 + r
    diff = (p - q).reshape(Bn, 16, 128, 4)
    jq = np.ascontiguousarray(diff.transpose(2, 0, 3, 1)).reshape(128, Bn * 64)
    return pt_s, qt_s, jq


def _get_nc(reps: int = 1):
    key = f"nc{reps}"
    if key not in _cache:
        _cache[key] = _build_bass(reps)
    return _cache[key]


def kernel(p: np.ndarray, q: np.ndarray, _trace: bool = False):
    from concourse.bass_utils import run_bass_kernel_spmd

    p = np.ascontiguousarray(np.asarray(p, dtype=np.float32))
    q = np.ascontiguousarray(np.asarray(q, dtype=np.float32))
    pt, qt, jq = _augment(p, q)
    jq3 = jq.reshape(128, B, 64)

    nc = _get_nc()
    in_maps = [
        {
            "pt": pt[c * BPC : (c + 1) * BPC],
            "qt": qt[c * BPC : (c + 1) * BPC],
            "jq": np.ascontiguousarray(jq3[:, c * BPC : (c + 1) * BPC].reshape(128, BPC * 64)),
        }
        for c in range(N_CORES)
    ]
    res = run_bass_kernel_spmd(nc, in_maps, list(range(N_CORES)), trace=_trace)
    total = float(np.sum([res.results[c]["out"][0, 0] for c in range(N_CORES)], dtype=np.float64))
    _cache["last_exec_time_ns"] = res.exec_time_ns
    return np.float32(total)

